# revision 17
# baseline (speedup 1.0000x reference)
"""Trainium2 Bass kernel for per-pixel cosine-distance block.

x1: [B, C, h, w]  f32
x2: [B, S, C, h, w] f32
out: [B, S*h*w] f32  where out[b, s*h*w + p] = 1 - cos(x1[b,:,p], x2[b,s,:,p])
(cosine over the channel dim C, per pixel)

Sharding: data-parallel over B across 8 NeuronCores (4 batches per core).

Per-core algorithm (C=512 on partitions as 4 chunks of 128, hw=1024 on free):
  dot[s,hw] = sum_c x1[c,hw] * x2[s,c,hw]   (DVE mult -> bf16, TensorE
                                             one-hot-matmul partition-reduce)
  ss2[s,hw] = sum_c x2[s,c,hw]^2            (ScalarE square -> bf16, matmul)
  ss1[hw]   = sum_c x1[c,hw]^2
  out = 1 - dot * rsqrt(ss1) * rsqrt(ss2)

The one-hot matmul (lhsT = e_s outer ones) accumulates each (s, chunk)
reduction into PSUM row s; dot rows live at partitions 0..7, ss2 at 32..39,
ss1 at 64 (engine PSUM access must be quadrant-based). rsqrt runs on the
Scalar engine via Abs_reciprocal_sqrt (ss >= 0), whose act table also holds
square/identity, so no table swaps. The epilogue of batch b is emitted inside
batch b+1's s-loop so no engine idles at batch boundaries.
"""

from contextlib import ExitStack

import numpy as np

import concourse.bass as bass
import concourse.tile as tile
from concourse import bacc, mybir
from concourse.bass_utils import run_bass_kernel_spmd

B, S, C, H, W = 32, 8, 512, 32, 32
HW = H * W  # 1024
N_CORES = 8
BL = B // N_CORES  # 4 batches per core
P = 128
NCH = C // P  # 4 chunks of the channel dim
HWH = HW // 2  # 512 (one PSUM bank of f32)

FP32 = mybir.dt.float32
BF16 = mybir.dt.bfloat16

# PSUM accumulator row layout (quadrant-based so engines can read each group)
D_ROW = 0  # rows 0..S-1: dot[s]
S2_ROW = 32  # rows 32..32+S-1: ss2[s]
S1_ROW = 64  # row 64: ss1
NR = S1_ROW + 1


def _emit(ctx: ExitStack, tc: tile.TileContext, x1, x2, out):
    nc = tc.nc

    # c = k*128 + p  ->  partition p, chunk k
    x1r = x1.rearrange("b (k p) f -> b p k f", p=P)  # [BL, 128, NCH, HW]
    x2r = x2.rearrange("b s (k p) f -> b s p k f", p=P)  # [BL, S, 128, NCH, HW]

    singles = ctx.enter_context(tc.tile_pool(name="singles", bufs=1))
    x1_pool = ctx.enter_context(tc.tile_pool(name="x1", bufs=2))
    x2_pool = ctx.enter_context(tc.tile_pool(name="x2", bufs=4))
    prod_pool = ctx.enter_context(tc.tile_pool(name="prod", bufs=3))
    sq1_pool = ctx.enter_context(tc.tile_pool(name="sq1", bufs=2))
    sq2_pool = ctx.enter_context(tc.tile_pool(name="sq2", bufs=3))
    ep_pool = ctx.enter_context(tc.tile_pool(name="ep", bufs=1))
    out_pool = ctx.enter_context(tc.tile_pool(name="outp", bufs=2))
    psum_pool = ctx.enter_context(tc.tile_pool(name="pacc", bufs=2, space="PSUM"))
    prep_pool = ctx.enter_context(tc.tile_pool(name="prep", bufs=2, space="PSUM"))

    # oh8[:, s, :] is a [P, S] matrix that is all-ones in column s, zero
    # elsewhere: matmul with it as lhsT deposits the partition-reduction of
    # rhs into PSUM row s of the output slice and adds zero to the others.
    oh8 = singles.tile([P, S, S], BF16)
    nc.vector.memset(oh8, 0.0)
    for s in range(S):
        nc.vector.memset(oh8[:, s, s : s + 1], 1.0)
    ones1 = singles.tile([P, 1], BF16)
    nc.vector.memset(ones1, 1.0)

    # [1, S] ones: K=1 matmul with it as lhsT replicates an SBUF row across
    # S PSUM partitions (fp32 so the values pass through unchanged).
    ones_row = singles.tile([1, S], FP32)
    nc.vector.memset(ones_row, 1.0)

    def emit_epilogue(pall, b):
        # out[b] = 1 - dot * rsqrt(ss1) * rsqrt(ss2)
        rr2 = ep_pool.tile([S, 2, HWH], FP32)
        nc.scalar.activation(
            rr2[:],
            pall[S2_ROW : S2_ROW + S],
            func=mybir.ActivationFunctionType.Abs_reciprocal_sqrt,
        )
        rr1 = ep_pool.tile([1, 2, HWH], FP32)
        nc.scalar.activation(
            rr1[:],
            pall[S1_ROW : S1_ROW + 1],
            func=mybir.ActivationFunctionType.Abs_reciprocal_sqrt,
        )
        # broadcast the rsqrt(ss1) row over the S s-partitions
        r1rep = prep_pool.tile([S, 2, HWH], FP32)  # 2 banks
        for hh in range(2):
            nc.tensor.matmul(
                r1rep[:, hh, :], ones_row, rr1[:, hh, :], start=True, stop=True
            )
        t = ep_pool.tile([S, 2, HWH], FP32)
        nc.vector.tensor_mul(t[:], pall[D_ROW : D_ROW + S], rr2[:])
        t2 = ep_pool.tile([S, 2, HWH], FP32)
        nc.vector.tensor_mul(t2[:], t[:], r1rep[:])
        dist = out_pool.tile([S, 2, HWH], FP32)
        # dist = 1 - t2, on the otherwise-idle GpSimd engine
        nc.gpsimd.tensor_scalar(
            dist[:],
            t2[:],
            -1.0,
            1.0,
            mybir.AluOpType.mult,
            mybir.AluOpType.add,
        )
        nc.gpsimd.dma_start(out[b], dist[:])

    pending = None
    x2_first = None
    for b in range(BL):
        if b == 0:
            # prefetch the first x2 tile so both DGE rings ramp immediately
            x2_first = x2_pool.tile([P, NCH, HW], FP32, tag="x2t")
            nc.gpsimd.dma_start(x2_first[:], x2r[0, 0])
        x1_t = x1_pool.tile([P, NCH, HW], FP32)
        nc.sync.dma_start(x1_t[:], x1r[b])

        sq1 = sq1_pool.tile([P, NCH, HW], BF16)
        # x1 squares on GpSimd: Scalar is near the critical path, GpSimd idle
        nc.gpsimd.tensor_mul(sq1[:], x1_t[:], x1_t[:])

        pall = psum_pool.tile([NR, 2, HWH], FP32)  # 2 banks

        for hh in range(2):
            for ic in range(NCH):
                nc.tensor.matmul(
                    pall[S1_ROW : S1_ROW + 1, hh, :],
                    ones1,
                    sq1[:, ic, hh * HWH : (hh + 1) * HWH],
                    start=(ic == 0),
                    stop=(ic == NCH - 1),
                )

        for s in range(S):
            if b == 0 and s == 0:
                x2_t = x2_first
            else:
                x2_t = x2_pool.tile([P, NCH, HW], FP32, tag="x2t")
                # rotate across the SP/Act HWDGE and gpsimd SWDGE rings so
                # transfers overlap across DMA-instruction boundaries
                dma_eng = (nc.sync, nc.gpsimd, nc.scalar)[(b * S + s) % 3]
                dma_eng.dma_start(x2_t[:], x2r[b, s])

            prod = prod_pool.tile([P, NCH, HW], BF16)
            nc.vector.tensor_mul(prod[:], x1_t[:], x2_t[:])
            sq2 = sq2_pool.tile([P, NCH, HW], BF16)
            nc.scalar.activation(
                sq2[:], x2_t[:], func=mybir.ActivationFunctionType.Square
            )

            for hh in range(2):
                for ic in range(NCH):
                    nc.tensor.matmul(
                        pall[D_ROW : D_ROW + S, hh, :],
                        oh8[:, s, :],
                        prod[:, ic, hh * HWH : (hh + 1) * HWH],
                        start=(s == 0 and ic == 0),
                        stop=(s == S - 1 and ic == NCH - 1),
                    )
                for ic in range(NCH):
                    nc.tensor.matmul(
                        pall[S2_ROW : S2_ROW + S, hh, :],
                        oh8[:, s, :],
                        sq2[:, ic, hh * HWH : (hh + 1) * HWH],
                        start=(s == 0 and ic == 0),
                        stop=(s == S - 1 and ic == NCH - 1),
                    )

            # previous batch's epilogue, pipelined into this batch's s-loop
            if s == 1 and pending is not None:
                emit_epilogue(*pending)
                pending = None

        pending = (pall, b)

    emit_epilogue(*pending)


def _build():
    # Bacc (not plain Bass): its compile pipeline legalizes TRN2's
    # one-sync-wait-per-instruction limit (generate_event_semaphores).
    nc = bacc.Bacc("TRN2")
    x1 = nc.dram_tensor("x1", [BL, C, HW], FP32, kind="ExternalInput")
    x2 = nc.dram_tensor("x2", [BL, S, C, HW], FP32, kind="ExternalInput")
    out = nc.dram_tensor("out", [BL, S, HW], FP32, kind="ExternalOutput")
    with tile.TileContext(nc) as tc:
        with ExitStack() as ctx:
            _emit(ctx, tc, x1[:], x2[:], out[:])
    nc.finalize()
    return nc


_NC = None

# test-harness knobs (the grading harness never touches these)
TRACE = False
TRACE_DIR = None
LAST_RESULTS = None


def _get_nc():
    global _NC
    if _NC is None:
        _NC = _build()
    return _NC


def kernel(x1: np.ndarray, x2: np.ndarray) -> np.ndarray:
    global LAST_RESULTS
    x1 = np.ascontiguousarray(x1, dtype=np.float32).reshape(B, C, HW)
    x2 = np.ascontiguousarray(x2, dtype=np.float32).reshape(B, S, C, HW)
    nc = _get_nc()
    in_maps = [
        {"x1": x1[c * BL : (c + 1) * BL], "x2": x2[c * BL : (c + 1) * BL]}
        for c in range(N_CORES)
    ]
    res = run_bass_kernel_spmd(
        nc, in_maps, list(range(N_CORES)), trace=TRACE, tmpdir=TRACE_DIR
    )
    LAST_RESULTS = res
    outs = [res.results[c]["out"].reshape(BL, S * HW) for c in range(N_CORES)]
    return np.concatenate(outs, axis=0)


# revision 18
# speedup vs baseline: 1.0727x; 1.0727x over previous
"""Trainium2 Bass kernel for per-pixel cosine-distance block.

x1: [B, C, h, w]  f32
x2: [B, S, C, h, w] f32
out: [B, S*h*w] f32  where out[b, s*h*w + p] = 1 - cos(x1[b,:,p], x2[b,s,:,p])
(cosine over the channel dim C, per pixel)

Sharding: data-parallel over B across 8 NeuronCores (4 batches per core).

Per-core algorithm (C=512 on partitions as 4 chunks of 128, hw=1024 on free):
  dot[s,hw] = sum_c x1[c,hw] * x2[s,c,hw]   (DVE mult -> bf16, TensorE
                                             one-hot-matmul partition-reduce)
  ss2[s,hw] = sum_c x2[s,c,hw]^2            (ScalarE square -> bf16, matmul)
  ss1[hw]   = sum_c x1[c,hw]^2
  out = 1 - dot * rsqrt(ss1) * rsqrt(ss2)

The one-hot matmul (lhsT = e_s outer ones) accumulates each (s, chunk)
reduction into PSUM row s; dot rows live at partitions 0..7, ss2 at 32..39,
ss1 at 64 (engine PSUM access must be quadrant-based). rsqrt runs on the
Scalar engine via Abs_reciprocal_sqrt (ss >= 0), whose act table also holds
square/identity, so no table swaps. The epilogue of batch b is emitted inside
batch b+1's s-loop so no engine idles at batch boundaries.
"""

from contextlib import ExitStack

import numpy as np

import concourse.bass as bass
import concourse.tile as tile
from concourse import bacc, mybir
from concourse.bass_utils import run_bass_kernel_spmd

B, S, C, H, W = 32, 8, 512, 32, 32
HW = H * W  # 1024
N_CORES = 8
BL = B // N_CORES  # 4 batches per core
P = 128
NCH = C // P  # 4 chunks of the channel dim
HWH = HW // 2  # 512 (one PSUM bank of f32)

FP32 = mybir.dt.float32
BF16 = mybir.dt.bfloat16

# PSUM accumulator row layout (quadrant-based so engines can read each group)
D_ROW = 0  # rows 0..S-1: dot[s]
S2_ROW = 32  # rows 32..32+S-1: ss2[s]
S1_ROW = 64  # row 64: ss1
NR = S1_ROW + 1


def _emit(ctx: ExitStack, tc: tile.TileContext, x1, x2, out):
    nc = tc.nc

    # c = k*128 + p  ->  partition p, chunk k
    x1r = x1.rearrange("b (k p) f -> b p k f", p=P)  # [BL, 128, NCH, HW]
    x2r = x2.rearrange("b s (k p) f -> b s p k f", p=P)  # [BL, S, 128, NCH, HW]

    singles = ctx.enter_context(tc.tile_pool(name="singles", bufs=1))
    x1_pool = ctx.enter_context(tc.tile_pool(name="x1", bufs=2))
    x2_pool = ctx.enter_context(tc.tile_pool(name="x2", bufs=4))
    prod_pool = ctx.enter_context(tc.tile_pool(name="prod", bufs=3))
    sq1_pool = ctx.enter_context(tc.tile_pool(name="sq1", bufs=2))
    sq2_pool = ctx.enter_context(tc.tile_pool(name="sq2", bufs=3))
    ep_pool = ctx.enter_context(tc.tile_pool(name="ep", bufs=2))
    out_pool = ctx.enter_context(tc.tile_pool(name="outp", bufs=2))
    psum_pool = ctx.enter_context(tc.tile_pool(name="pacc", bufs=2, space="PSUM"))
    prep_pool = ctx.enter_context(tc.tile_pool(name="prep", bufs=2, space="PSUM"))

    # oh8[:, s, :] is a [P, S] matrix that is all-ones in column s, zero
    # elsewhere: matmul with it as lhsT deposits the partition-reduction of
    # rhs into PSUM row s of the output slice and adds zero to the others.
    oh8 = singles.tile([P, S, S], BF16)
    nc.vector.memset(oh8, 0.0)
    for s in range(S):
        nc.vector.memset(oh8[:, s, s : s + 1], 1.0)
    ones1 = singles.tile([P, 1], BF16)
    nc.vector.memset(ones1, 1.0)

    # [1, S] ones: K=1 matmul with it as lhsT replicates an SBUF row across
    # S PSUM partitions (fp32 so the values pass through unchanged).
    ones_row = singles.tile([1, S], FP32)
    nc.vector.memset(ones_row, 1.0)

    def emit_epilogue(pall, b):
        # out[b] = 1 - dot * rsqrt(ss1) * rsqrt(ss2)
        rr2 = ep_pool.tile([S, 2, HWH], FP32)
        nc.scalar.activation(
            rr2[:],
            pall[S2_ROW : S2_ROW + S],
            func=mybir.ActivationFunctionType.Abs_reciprocal_sqrt,
        )
        rr1 = ep_pool.tile([1, 2, HWH], FP32)
        nc.scalar.activation(
            rr1[:],
            pall[S1_ROW : S1_ROW + 1],
            func=mybir.ActivationFunctionType.Abs_reciprocal_sqrt,
        )
        # broadcast the rsqrt(ss1) row over the S s-partitions
        r1rep = prep_pool.tile([S, 2, HWH], FP32)  # 2 banks
        for hh in range(2):
            nc.tensor.matmul(
                r1rep[:, hh, :], ones_row, rr1[:, hh, :], start=True, stop=True
            )
        t = ep_pool.tile([S, 2, HWH], FP32)
        nc.vector.tensor_mul(t[:], pall[D_ROW : D_ROW + S], rr2[:])
        t2 = ep_pool.tile([S, 2, HWH], FP32)
        nc.vector.tensor_mul(t2[:], t[:], r1rep[:])
        dist = out_pool.tile([S, 2, HWH], FP32)
        # dist = 1 - t2, on the otherwise-idle GpSimd engine
        nc.gpsimd.tensor_scalar(
            dist[:],
            t2[:],
            -1.0,
            1.0,
            mybir.AluOpType.mult,
            mybir.AluOpType.add,
        )
        nc.gpsimd.dma_start(out[b], dist[:])

    pending = None
    for b in range(BL):
        x1_t = x1_pool.tile([P, NCH, HW], FP32)
        nc.sync.dma_start(x1_t[:], x1r[b])

        sq1 = sq1_pool.tile([P, NCH, HW], BF16)
        # x1 squares on GpSimd: Scalar is near the critical path, GpSimd idle
        nc.gpsimd.tensor_mul(sq1[:], x1_t[:], x1_t[:])

        pall = psum_pool.tile([NR, 2, HWH], FP32)  # 2 banks

        for hh in range(2):
            for ic in range(NCH):
                nc.tensor.matmul(
                    pall[S1_ROW : S1_ROW + 1, hh, :],
                    ones1,
                    sq1[:, ic, hh * HWH : (hh + 1) * HWH],
                    start=(ic == 0),
                    stop=(ic == NCH - 1),
                )

        for s in range(S):
            x2_t = x2_pool.tile([P, NCH, HW], FP32)
            # alternate HWDGE (SP) and SWDGE (gpsimd) rings so transfers of
            # consecutive tiles overlap across DMA-instruction boundaries
            dma_eng = nc.sync if s % 2 == 0 else nc.gpsimd
            dma_eng.dma_start(x2_t[:], x2r[b, s])

            prod = prod_pool.tile([P, NCH, HW], BF16)
            nc.vector.tensor_mul(prod[:], x1_t[:], x2_t[:])
            sq2 = sq2_pool.tile([P, NCH, HW], BF16)
            nc.scalar.activation(
                sq2[:], x2_t[:], func=mybir.ActivationFunctionType.Square
            )

            for hh in range(2):
                for ic in range(NCH):
                    nc.tensor.matmul(
                        pall[D_ROW : D_ROW + S, hh, :],
                        oh8[:, s, :],
                        prod[:, ic, hh * HWH : (hh + 1) * HWH],
                        start=(s == 0 and ic == 0),
                        stop=(s == S - 1 and ic == NCH - 1),
                    )
                for ic in range(NCH):
                    nc.tensor.matmul(
                        pall[S2_ROW : S2_ROW + S, hh, :],
                        oh8[:, s, :],
                        sq2[:, ic, hh * HWH : (hh + 1) * HWH],
                        start=(s == 0 and ic == 0),
                        stop=(s == S - 1 and ic == NCH - 1),
                    )

            # previous batch's epilogue, pipelined into this batch's s-loop
            if s == 1 and pending is not None:
                emit_epilogue(*pending)
                pending = None

        pending = (pall, b)

    emit_epilogue(*pending)


def _build():
    # Bacc (not plain Bass): its compile pipeline legalizes TRN2's
    # one-sync-wait-per-instruction limit (generate_event_semaphores).
    nc = bacc.Bacc("TRN2")
    x1 = nc.dram_tensor("x1", [BL, C, HW], FP32, kind="ExternalInput")
    x2 = nc.dram_tensor("x2", [BL, S, C, HW], FP32, kind="ExternalInput")
    out = nc.dram_tensor("out", [BL, S, HW], FP32, kind="ExternalOutput")
    with tile.TileContext(nc) as tc:
        with ExitStack() as ctx:
            _emit(ctx, tc, x1[:], x2[:], out[:])
    nc.finalize()
    return nc


_NC = None

# test-harness knobs (the grading harness never touches these)
TRACE = False
TRACE_DIR = None
LAST_RESULTS = None


def _get_nc():
    global _NC
    if _NC is None:
        _NC = _build()
    return _NC


def kernel(x1: np.ndarray, x2: np.ndarray) -> np.ndarray:
    global LAST_RESULTS
    x1 = np.ascontiguousarray(x1, dtype=np.float32).reshape(B, C, HW)
    x2 = np.ascontiguousarray(x2, dtype=np.float32).reshape(B, S, C, HW)
    nc = _get_nc()
    in_maps = [
        {"x1": x1[c * BL : (c + 1) * BL], "x2": x2[c * BL : (c + 1) * BL]}
        for c in range(N_CORES)
    ]
    res = run_bass_kernel_spmd(
        nc, in_maps, list(range(N_CORES)), trace=TRACE, tmpdir=TRACE_DIR
    )
    LAST_RESULTS = res
    outs = [res.results[c]["out"].reshape(BL, S * HW) for c in range(N_CORES)]
    return np.concatenate(outs, axis=0)
